# revision 14
# baseline (speedup 1.0000x reference)
"""Trainium2 Bass kernel for causal MultiHeadAttention.

Problem: x[4, 2048, 768], 12 heads x 64 dim, causal, scale = 768**-0.5,
y = softmax(mask(q @ k.T * scale)) @ v  (concat heads) @ Wp + bp.

The wall-clock of run_bass_kernel_spmd is dominated by the ~45 MB/s axon
tunnel, so the design minimizes bytes on the wire:

  * Sharding: core c = (batch b = c//2, query-half hf = c%2).  Queries are
    interleaved at 128-row block granularity (core's local block j is
    global block 2j+hf) so the causal structure -- and hence the SPMD
    program -- is identical on every core.
  * Each core receives only its own 1024 query rows of x[b] (bf16) and a
    1/8 row-shard of the packed weights (bf16).  On-device AllGather
    rebuilds x[b] within core pairs and the full weight matrix across all
    8 cores: every input byte crosses the tunnel exactly once.
  * Each core returns only its own 1024 output rows in bf16 (the host
    upcasts to f32), so output bytes also cross the tunnel once.

Per-core dataflow (matmuls bf16 -> PSUM f32):
  1. pair-AllGather xq -> xg [2048,768] (even|odd block order); 8-core
     AllGather of the weight shard -> W_all [3072,768] = [Wq;Wk;Wv;Wp].
  2. PE-transpose xq and xg; project QT [hd,1024], KT [hd,2048] (gathered
     order), and Vaug [s, 16 x (12 heads x 65)] where column 64 of each
     head slot is ones so the PV matmul also emits softmax denominators.
  3. per head h, per 512-query chunk c: for key block sb < 8c+8:
     ST[s,q] = KT_sb^T @ QT_chunk (query range trimmed to the causal
     staircase), P = exp(ST*scale) with a zero prefix, boundary blocks
     sb in {2j, 2j+1} multiplied by the per-core mask pair m2
     ([tril|zeros] on even-half cores, [ones|tril] on odd), then
     OT[65,512] += Vaug_sb^T @ P.  Row 64 of OT holds denominators;
     normalize rows 0:64 into otall2.
  4. y[q,e] = otall2^T @ Wp + bp -> DMA bf16 to DRAM.
"""

import os
import sys

if "/opt/trn_rl_repo" not in sys.path:
    sys.path.insert(0, "/opt/trn_rl_repo")

# Smaller NEFF (no debug info) -> slightly faster walrus + executable load.
os.environ.setdefault("CONCOURSE_SCRUB_NEFF_DEBUG_INFO", "1")

import numpy as np
import ml_dtypes

import concourse.bass as bass
import concourse.mybir as mybir
import concourse.tile as tile
from concourse.bass_utils import run_bass_kernel_spmd

# ---------------------------------------------------------------------------
# The default DVE table set is a pure function of (arch, {}) but is rebuilt
# (deepcopy of the stock base image + repacking) on every compile -- ~0.5s
# per run_bass_kernel_spmd call.  Memoize it.
import concourse.bass_utils as _bu
import concourse.dve_table_gen as _dtg

_orig_gen_dve = _dtg.generate_dve_tables
_dve_memo = {}


def _memo_gen_dve(trn_type, ops, base_dir=None):
    if ops or base_dir is not None:
        return _orig_gen_dve(trn_type, ops, base_dir)
    if trn_type not in _dve_memo:
        _dve_memo[trn_type] = _orig_gen_dve(trn_type, ops, base_dir)
    return dict(_dve_memo[trn_type])


_dtg.generate_dve_tables = _memo_gen_dve
_bu.generate_dve_tables = _memo_gen_dve
# ---------------------------------------------------------------------------

# ---------------------------------------------------------------------------
# This walrus build rejects instructions carrying more than one sem wait
# ("Too many sync wait commands" in setupSyncWait).  Post-pass: move excess
# waits onto preceding same-engine NoOps (the engine stalls identically).
_MAXW = 1


def _split_waits(nc):
    for fn in nc.m.functions:
        for bb in fn.blocks:
            out = []
            for inst in bb.instructions:
                si = getattr(inst, "sync_info", None)
                if (
                    si is not None
                    and si.on_wait
                    and len(si.on_wait) > _MAXW
                    and inst.opcode != "EventSemaphore"
                ):
                    waits = list(si.on_wait)
                    for k, i0 in enumerate(range(_MAXW, len(waits), _MAXW)):
                        out.append(mybir.InstNoOp(
                            name=f"{inst.name}_xw{k}",
                            engine=inst.engine,
                            sync_info=mybir.SyncInfo(
                                on_wait=waits[i0 : i0 + _MAXW], on_update=[]
                            ),
                            bass_nofuse=True,
                        ))
                    inst.sync_info = mybir.SyncInfo(
                        on_wait=waits[:_MAXW], on_update=list(si.on_update)
                    )
                out.append(inst)
            bb.instructions = out
# ---------------------------------------------------------------------------

F32 = mybir.dt.float32
F32R = mybir.dt.float32r
BF16 = mybir.dt.bfloat16
EXP = mybir.ActivationFunctionType.Exp
BF16NP = ml_dtypes.bfloat16

B, T, C = 4, 2048, 768
H, D = 12, 64
N_CORES = 8
TQ = T // 2           # queries per core
NQB = TQ // 128       # local query blocks (8)
NSB = T // 128        # key blocks (16)
NCC = C // 128        # contraction chunks (6)
SCALE = float(C) ** -0.5


def r(ap):
    return ap.bitcast(F32R)


def _gmap(g):
    """global 128-block index -> row-block in the pair-gathered x."""
    return g // 2 if g % 2 == 0 else NQB + g // 2


def build_nc():
    nc = bass.Bass("TRN2", target_bir_lowering=False, debug=False,
                   num_devices=N_CORES)
    xq_d = nc.dram_tensor("xq", [TQ, C], BF16, kind="ExternalInput")
    wsh_d = nc.dram_tensor("wsh", [4 * C // N_CORES, C], BF16,
                           kind="ExternalInput")
    m2_d = nc.dram_tensor("m2", [128, 256], BF16, kind="ExternalInput")
    ident_d = nc.dram_tensor("ident", [128, 128], BF16, kind="ExternalInput")
    bp_d = nc.dram_tensor("bp", [1, C], F32R, kind="ExternalInput")
    y_d = nc.dram_tensor("y", [TQ, C], BF16, kind="ExternalOutput")

    with tile.TileContext(nc) as tc:
        with (
            tc.tile_pool(name="dram", bufs=1, space="DRAM") as dram,
            tc.tile_pool(name="persist", bufs=1) as pp,
        ):
            # ---- bounce buffers + collectives ----
            wsh_b = dram.tile([4 * C // N_CORES, C], BF16, name="wsh_b")
            w_full = dram.tile([4 * C, C], BF16, name="w_full",
                               addr_space="Shared")
            xq_b = dram.tile([TQ, C], BF16, name="xq_b")
            xg = dram.tile([T, C], BF16, name="xg")
            nc.gpsimd.dma_start(wsh_b[:], wsh_d[:])
            nc.gpsimd.dma_start(xq_b[:], xq_d[:])
            nc.gpsimd.collective_compute(
                "AllGather", mybir.AluOpType.bypass,
                replica_groups=[list(range(N_CORES))],
                ins=[wsh_b.opt()], outs=[w_full.opt()],
            )
            nc.gpsimd.collective_compute(
                "AllGather", mybir.AluOpType.bypass,
                replica_groups=[[0, 1], [2, 3], [4, 5], [6, 7]],
                ins=[xq_b.opt()], outs=[xg.opt()],
            )

            # ---- persistent SBUF ----
            ident = pp.tile([128, 128], BF16, name="ident", tag="ident")
            nc.sync.dma_start(ident[:], ident_d[:])
            m2 = pp.tile([128, 256], BF16, name="m2", tag="m2")
            nc.sync.dma_start(m2[:], m2_d[:])
            bp_sb = pp.tile([1, C], F32R, name="bp_sb", tag="bp_sb")
            nc.sync.dma_start(bp_sb[:], bp_d[:])

            czero = pp.tile([128, 384], BF16, name="czero", tag="czero")
            nc.gpsimd.memset(czero[:], 0.0)
            cone = pp.tile([128, H], BF16, name="cone", tag="cone")
            nc.gpsimd.memset(cone[:], 1.0)
            # fp32r ones (walrus rejects memset on fp32r; copy from f32)
            scr1 = pp.tile([1, 128], F32, name="scr1", tag="scr1")
            nc.gpsimd.memset(scr1[:], 1.0)
            ones64 = pp.tile([1, 64], F32R, name="ones64", tag="ones64")
            nc.vector.tensor_copy(ones64[:], scr1[:, 0:64])
            ones128 = pp.tile([1, 128], F32R, name="ones128", tag="ones128")
            nc.vector.tensor_copy(ones128[:], scr1[:])

            # weights in SBUF: w_sb[w][cc] = W_all[w*768 + cc*128 :][:128]
            w_sb = [
                [pp.tile([128, C], BF16, name=f"w{w}_{cc}", tag=f"w{w}_{cc}")
                 for cc in range(NCC)]
                for w in range(4)
            ]
            for w in range(4):
                for cc in range(NCC):
                    nc.sync.dma_start(
                        w_sb[w][cc][:],
                        w_full[w * C + cc * 128 : w * C + (cc + 1) * 128, :],
                    )

            # bias broadcast tile [128, 768]
            bpb = pp.tile([128, C], F32, name="bpb", tag="bpb")
            with tc.tile_pool(name="bpp", bufs=1, space="PSUM") as bppp:
                for mv in range(2):
                    sl = slice(mv * 384, (mv + 1) * 384)
                    bpp = bppp.tile([128, 384], F32, name="bpp", tag="bpp")
                    nc.tensor.matmul(bpp[:], ones128[:], bp_sb[0:1, sl],
                                     start=True, stop=True)
                    nc.scalar.copy(bpb[:, sl], bpp[:])

            xqt = [pp.tile([128, TQ], BF16, name=f"xqt{i}", tag=f"xqt{i}")
                   for i in range(NCC)]
            xt = [pp.tile([128, T], BF16, name=f"xt{i}", tag=f"xt{i}")
                  for i in range(NCC)]
            qt = pp.tile([128, 6 * TQ], BF16, name="qt", tag="qt")
            kt = pp.tile([128, 6 * T], BF16, name="kt", tag="kt")
            vaug = pp.tile([128, NSB * H * 65], BF16, name="vaug", tag="vaug")
            otall = pp.tile([128, 6 * TQ], BF16, name="otall", tag="otall")

            # ---- transpose xq and xg ----
            with (
                tc.tile_pool(name="xst", bufs=4) as xsp,
                tc.tile_pool(name="tps", bufs=2, space="PSUM") as tpp,
            ):
                for src, tgt, nblk in ((xq_d, xqt, NQB), (xg, xt, NSB)):
                    for tcg in range(nblk // 4):
                        xtiles = []
                        for i in range(4):
                            tb = tcg * 4 + i
                            xs = xsp.tile([128, C], BF16, name="xs", tag="xs")
                            nc.sync.dma_start(
                                xs[:], src[tb * 128 : (tb + 1) * 128, :])
                            xtiles.append(xs)
                        for cc in range(NCC):
                            tp = tpp.tile([128, 512], BF16, name="tp", tag="tp")
                            for i in range(4):
                                nc.tensor.transpose(
                                    tp[:, i * 128 : (i + 1) * 128],
                                    xtiles[i][:, cc * 128 : (cc + 1) * 128],
                                    ident[:],
                                )
                            nc.vector.tensor_copy(
                                tgt[cc][:, tcg * 512 : (tcg + 1) * 512],
                                tp[:],
                            )

            # ---- project QT, KT (gathered order), Vaug ----
            with tc.tile_pool(name="qkps", bufs=3, space="PSUM") as qkp:
                for w, tgt, src, tlen in ((0, qt, xqt, TQ), (1, kt, xt, T)):
                    for hp in range(NCC):
                        for tcg in range(tlen // 512):
                            mm = qkp.tile([128, 512], F32, name="mm", tag="mm")
                            for cc in range(NCC):
                                nc.tensor.matmul(
                                    mm[:],
                                    w_sb[w][cc][:, hp * 128 : (hp + 1) * 128],
                                    src[cc][:, tcg * 512 : (tcg + 1) * 512],
                                    start=(cc == 0), stop=(cc == NCC - 1),
                                )
                            nc.vector.tensor_copy(
                                tgt[:, hp * tlen + tcg * 512
                                    : hp * tlen + (tcg + 1) * 512],
                                mm[:],
                            )
            with tc.tile_pool(name="vps", bufs=4, space="PSUM") as vpp:
                for sb in range(NSB):
                    va = vaug[:, sb * H * 65 : (sb + 1) * H * 65].rearrange(
                        "p (h e) -> p h e", e=65)
                    nc.vector.tensor_copy(va[:, :, 64:65], cone[:].unsqueeze(2))
                    for mv in range(2):
                        sl = slice(mv * 384, (mv + 1) * 384)
                        vp = vpp.tile([128, 384], F32, name="vp", tag="vp")
                        for cc in range(NCC):
                            nc.tensor.matmul(
                                vp[:],
                                xt[cc][:, sb * 128 : (sb + 1) * 128],
                                w_sb[2][cc][:, sl],
                                start=(cc == 0), stop=(cc == NCC - 1),
                            )
                        nc.scalar.copy(
                            va[:, mv * 6 : (mv + 1) * 6, 0:64],
                            vp.rearrange("p (h e) -> p h e", e=64))

            # ---- attention ----
            with (
                tc.tile_pool(name="stps", bufs=3, space="PSUM") as stp,
                tc.tile_pool(name="otps", bufs=2, space="PSUM") as otp,
                tc.tile_pool(name="bcps", bufs=2, space="PSUM") as bcp_p,
                tc.tile_pool(name="pts", bufs=3) as ptp,
                tc.tile_pool(name="small", bufs=2) as sp,
            ):
                for h in range(H):
                    hp, dlt = h // 2, (h % 2) * 64
                    prow = slice(dlt, dlt + 64)
                    for c in range(2):
                        q0 = hp * TQ + c * 512
                        n_sb = 8 * c + 8
                        ot = otp.tile([65, 512], F32, name="ot", tag="ot")
                        for sb in range(n_sb):
                            j_min = max(0, -(-(sb - 1) // 2))
                            off = 128 * max(0, j_min - 4 * c)
                            mb = _gmap(sb)
                            st = stp.tile([128, 512], F32, name="st", tag="st")
                            nc.tensor.matmul(
                                st[:, off:512],
                                kt[prow, hp * T + mb * 128
                                   : hp * T + (mb + 1) * 128],
                                qt[prow, q0 + off : q0 + 512],
                                start=True, stop=True,
                            )
                            pt = ptp.tile([128, 512], BF16, name="pt", tag="pt")
                            if off:
                                nc.vector.tensor_copy(pt[:, 0:off],
                                                      czero[:, 0:off])
                            nc.scalar.activation(
                                pt[:, off:512], st[:, off:512], EXP,
                                scale=SCALE)
                            if sb >= 8 * c:
                                qoff = (sb // 2 - 4 * c) * 128
                                nc.vector.tensor_mul(
                                    pt[:, qoff : qoff + 128],
                                    pt[:, qoff : qoff + 128],
                                    m2[:, (sb % 2) * 128 : (sb % 2 + 1) * 128],
                                )
                            nc.tensor.matmul(
                                ot[:],
                                vaug[:, mb * H * 65 + h * 65
                                     : mb * H * 65 + h * 65 + 65],
                                pt[:],
                                start=(sb == 0), stop=(sb == n_sb - 1),
                            )
                        rt = sp.tile([1, 512], F32R, name="rt", tag="rt")
                        with nc.allow_low_precision(reason="f32r is 32-bit"):
                            nc.vector.reciprocal(rt[:], ot[64:65, :])
                        bcp = bcp_p.tile([64, 512], F32, name="bcp", tag="bcp")
                        nc.tensor.matmul(bcp[:], ones64[:], rt[:],
                                         start=True, stop=True)
                        bcs = sp.tile([64, 512], F32, name="bcs", tag="bcs")
                        nc.scalar.copy(bcs[:], bcp[:])
                        nc.vector.tensor_mul(
                            otall[prow, q0 : q0 + 512], ot[0:64, :], bcs[:])

            # ---- output projection + bias ----
            with (
                tc.tile_pool(name="yps", bufs=4, space="PSUM") as ypp,
                tc.tile_pool(name="ysb", bufs=4) as ysp,
            ):
                for tb in range(NQB):
                    for eh in range(2):
                        sl = slice(eh * 384, (eh + 1) * 384)
                        yp = ypp.tile([128, 384], F32, name="yp", tag="yp")
                        for hp in range(NCC):
                            nc.tensor.matmul(
                                yp[:],
                                otall[:, hp * TQ + tb * 128
                                      : hp * TQ + (tb + 1) * 128],
                                w_sb[3][hp][:, sl],
                                start=(hp == 0), stop=(hp == NCC - 1),
                            )
                        ys = ysp.tile([128, 384], BF16, name="ys", tag="ys")
                        nc.vector.tensor_add(ys[:], yp[:], bpb[:, sl])
                        nc.sync.dma_start(
                            y_d[tb * 128 : (tb + 1) * 128, sl], ys[:])
    _split_waits(nc)
    return nc


_NC_CACHE = {}


def _get_nc(t=T):
    if t not in _NC_CACHE:
        _NC_CACHE[t] = build_nc()
    return _NC_CACHE[t]


def _shard_inputs(x, Wq, Wk, Wv, Wp):
    x = np.asarray(x, dtype=np.float32)
    w2d = [np.transpose(np.asarray(w), (1, 0, 2)).reshape(C, C)
           for w in (Wq, Wk, Wv)]
    w_all = np.concatenate(w2d + [np.asarray(Wp)], axis=0).astype(BF16NP)
    tril = (np.arange(128)[:, None] <= np.arange(128)[None, :])
    ones = np.ones((128, 128), np.float32)
    zeros = np.zeros((128, 128), np.float32)
    m2s = [np.concatenate([tril * ones, zeros], axis=1).astype(BF16NP),
           np.concatenate([ones, tril * ones], axis=1).astype(BF16NP)]
    shard = 4 * C // N_CORES
    in_maps = []
    for core in range(N_CORES):
        b, hf = core // 2, core % 2
        xq = np.ascontiguousarray(
            x[b].reshape(NSB, 128, C)[hf::2].reshape(TQ, C)).astype(BF16NP)
        in_maps.append({
            "xq": xq,
            "wsh": np.ascontiguousarray(w_all[core * shard:(core + 1) * shard]),
            "m2": m2s[hf],
            "ident": np.eye(128, dtype=BF16NP),
            "bp": np.zeros((1, C), np.float32),
        })
    return in_maps


def kernel(x, Wq, Wk, Wv, Wp, bp, mask):
    assert mask, "kernel hardcodes causal masking"
    nc = _get_nc(T)
    in_maps = _shard_inputs(x, Wq, Wk, Wv, Wp)
    bp = np.asarray(bp, dtype=np.float32).reshape(1, C)
    for m in in_maps:
        m["bp"] = bp
    res = run_bass_kernel_spmd(nc, in_maps, list(range(N_CORES)))
    out = np.empty((B, T, C), dtype=np.float32)
    ov = out.reshape(B, NSB, 128, C)
    for core in range(N_CORES):
        b, hf = core // 2, core % 2
        ov[b, hf::2] = res.results[core]["y"].reshape(
            NQB, 128, C).astype(np.float32)
    return out
